# revision 38
# baseline (speedup 1.0000x reference)
"""BipartiteGCN message-passing kernel for 8 TRN2 NeuronCores.

Math:  out = D_c^{-1/2} A^T D_r^{-1/2} (x @ W) + b
where A[s, d] = multiplicity of edge (gene s, drug d), s, d in [0, 4000).

Strategy (gene sharding + one f16 ReduceScatter):
  - Core c owns gene range [512c, 512c+512) and ALL drug columns.  The
    host groups that core's edges by (gene row, 128-gene window, dst
    quarter), dedupes per (gene,dst) pair into (dst-index, multiplicity)
    slots (pure index layout, no float arithmetic).
  - Each [128 x 1024] block of the dense count stripe A (f16) is built
    in SBUF by ONE gpsimd local_scatter (per-partition indices,
    multiplicity payload).  No one-hot matmuls, no gather/scatter DMA.
  - row_deg is fully local (a core owns complete gene rows): one
    f16 reduce over the multiplicity table.  f = rsqrt(row_deg)
    scales A in place.  No AllReduce.
  - col_deg rides along as payload column 512 of the partial GEMM
    output: tiny rank-1 matmuls (f*A)^T @ (1/f) accumulate per-dst
    column sums in PSUM.
  - xw = x_shard @ W in f32r (1 cycle/row), cast to f16.
  - partials[d, :512] = (f*A)^T @ xw over the 512 local genes; one
    ReduceScatter(add) over the [4096, 513] f16 partials both sums the
    8 per-core partials and hands core c exactly its 512-drug window.
  - out = g * partial + bias ; host concatenates the dst stripes.
"""

import sys

if "/opt/trn_rl_repo" not in sys.path:
    sys.path.insert(0, "/opt/trn_rl_repo")

import numpy as np

import concourse.bass as bass  # noqa: F401
import concourse.mybir as mybir
from concourse import bacc, tile

CORES = 8
GSH = 512               # genes per core
NWJ = GSH // 128        # 4 local gene windows
ND = 4000               # number of drugs
NDP = 4096              # padded drug dim
NQ = 4                  # dst quarters
QW = NDP // NQ          # 1024 dst per quarter
IC = 1024
OC = 512
PC = 513                # RS payload columns: 512 out + col_deg
KMAX = 48               # max deduped slots per (gene row, window, quarter)

F32 = mybir.dt.float32
F32R = mybir.dt.float32r
F16 = mybir.dt.float16
I16 = mybir.dt.int16


def build_nc(debug_outputs=False):
    nc = bacc.Bacc(
        None,
        target_bir_lowering=False,
        debug=False,
        num_devices=CORES,
    )

    # xh[p, k, gl] = x[512c + gl, 128k + p]  (lhsT layout for x @ W)
    xh = nc.dram_tensor("xh", [128, IC // 128, GSH], F32R, kind="ExternalInput")
    wh = nc.dram_tensor("wh", [128, IC // 128, OC], F32R, kind="ExternalInput")
    brep = nc.dram_tensor("brep", [128, OC], F16, kind="ExternalInput")
    # slot tables: col = j*(NQ*KMAX) + q*KMAX + k
    idxt = nc.dram_tensor("idxt", [128, NWJ * NQ * KMAX], I16, kind="ExternalInput")
    datt = nc.dram_tensor("datt", [128, NWJ, NQ * KMAX], F16, kind="ExternalInput")
    out = nc.dram_tensor("out", [GSH, OC], F16, kind="ExternalOutput")

    rsin = nc.dram_tensor("rsin", [NDP, PC], F16)
    rsout = nc.dram_tensor("rsout", [GSH, PC], F16)

    with tile.TileContext(nc) as tc:
        with (
            tc.tile_pool(name="const", bufs=1) as cpool,
            tc.tile_pool(name="work", bufs=2) as wpool,
            tc.tile_pool(name="apool", bufs=NWJ * NQ) as apool,
            tc.tile_pool(name="psum", bufs=4, space="PSUM") as ppool,
        ):
            # --- input loads, ordered for the critical path: index
            # tables gate the Pool scatters; x windows + W gate the PE --
            idx_sb = cpool.tile([128, NWJ * NQ * KMAX], I16)
            nc.sync.dma_start(idx_sb[:], idxt[:])
            dat_sb = cpool.tile([128, NWJ, NQ * KMAX], F16)
            nc.sync.dma_start(dat_sb[:], datt[:])
            x_sb = cpool.tile([128, IC // 128, GSH], F32R)
            nc.sync.dma_start(x_sb[:, :, 0:128], xh[:, :, 0:128])
            w_sb = cpool.tile([128, IC // 128, OC], F32R)
            nc.sync.dma_start(w_sb[:], wh[:])
            for j in range(1, NWJ):
                nc.sync.dma_start(
                    x_sb[:, :, j * 128:(j + 1) * 128],
                    xh[:, :, j * 128:(j + 1) * 128],
                )
            brep_sb = cpool.tile([128, OC], F16)
            nc.sync.dma_start(brep_sb[:], brep[:])

            # --- row_deg (local!) + f, 1/f ----------------------------
            rd16 = cpool.tile([128, NWJ], F16)
            with nc.allow_low_precision("int-valued degree sums <2048 exact in f16"):
                nc.vector.reduce_sum(rd16[:], dat_sb[:], axis=mybir.AxisListType.X)
            # rows with deg 0 have all-zero A rows, so no rsqrt masking is
            # needed: f and 1/f multiply zeros.
            t1 = cpool.tile([128, NWJ], F32)
            nc.vector.tensor_scalar(
                out=t1[:], in0=rd16[:], scalar1=1.0, scalar2=None,
                op0=mybir.AluOpType.max,
            )
            nc.scalar.sqrt(t1[:], t1[:])
            f_sb = cpool.tile([128, NWJ], F32)
            nc.vector.reciprocal(f_sb[:], t1[:])
            finv16 = cpool.tile([128, NWJ], F16)
            nc.vector.tensor_copy(finv16[:], t1[:])

            # --- A blocks: one local_scatter per (quarter, window) ----
            a_sb = {}
            for q in range(NQ):
                for j in range(NWJ):
                    a_t = apool.tile([128, QW], F16, tag="A", name=f"a{q}_{j}")
                    base = j * (NQ * KMAX) + q * KMAX
                    nc.gpsimd.local_scatter(
                        out_ap=a_t[:],
                        data_ap=dat_sb[:, j, q * KMAX:(q + 1) * KMAX],
                        idxs_ap=idx_sb[:, base:base + KMAX],
                        channels=128,
                        num_elems=QW,
                        num_idxs=KMAX,
                    )
                    a_sb[(q, j)] = a_t

            # --- xw = x_shard @ W (f32r), cast to f16 -----------------
            xw_sb = []
            for j in range(NWJ):
                pxw = ppool.tile([128, OC], F32, tag="xwp", bufs=2, name=f"pxw{j}")
                for k in range(IC // 128):
                    nc.tensor.matmul(
                        pxw[:],
                        x_sb[:, k, j * 128:(j + 1) * 128],
                        w_sb[:, k, :],
                        start=(k == 0),
                        stop=(k == IC // 128 - 1),
                    )
                xw_t = cpool.tile([128, OC], F16, tag="XW", bufs=NWJ,
                                  name=f"xw{j}")
                nc.scalar.copy(xw_t[:], pxw[:])
                xw_sb.append(xw_t)

            # --- per quarter: scale A rows by f in place, then GEMM ---
            # (scale interleaved with the GEMM so the in-order Act/DVE
            # queues never hold a later quarter's work in front of this
            # quarter's PSUM flushes)
            for q in range(NQ):
                for j in range(NWJ):
                    if j % 2 == 0:
                        nc.vector.tensor_scalar(
                            out=a_sb[(q, j)][:], in0=a_sb[(q, j)][:],
                            scalar1=f_sb[:, j:j + 1], scalar2=None,
                            op0=mybir.AluOpType.mult,
                        )
                    else:
                        nc.scalar.mul(
                            a_sb[(q, j)][:], a_sb[(q, j)][:], f_sb[:, j:j + 1]
                        )
                for t in range(QW // 128):
                    pt = q * (QW // 128) + t
                    pp = ppool.tile([128, OC], F32, tag="pp", bufs=3,
                                    name=f"pp{pt}")
                    pcd = ppool.tile([128, OC], F32, tag="cd", bufs=2,
                                     name=f"pcd{pt}")
                    for j in range(NWJ):
                        nc.tensor.matmul(
                            pp[:],
                            a_sb[(q, j)][:, t * 128:(t + 1) * 128],
                            xw_sb[j][:],
                            start=(j == 0),
                            stop=(j == NWJ - 1),
                        )
                        nc.tensor.matmul(
                            pcd[:, 0:1],
                            a_sb[(q, j)][:, t * 128:(t + 1) * 128],
                            finv16[:, j:j + 1],
                            start=(j == 0),
                            stop=(j == NWJ - 1),
                        )
                    st = wpool.tile([128, PC], F16, tag="st", bufs=5,
                                    name=f"st{pt}")
                    if pt % 2 == 0:
                        nc.scalar.copy(st[:, 0:OC], pp[:])
                    else:
                        nc.vector.tensor_copy(st[:, 0:OC], pp[:])
                    nc.vector.tensor_copy(st[:, OC:OC + 1], pcd[:, 0:1])
                    nc.sync.dma_start(rsin[pt * 128:(pt + 1) * 128, :], st[:])

            # --- ReduceScatter: sum partials, keep my dst window ------
            # (hand-built with unflattened row-major APs; .opt() would merge
            # the AP into one flat run)
            from concourse.replica_groups import filter_and_check_groups
            rgs = filter_and_check_groups(CORES, [list(range(CORES))])
            nc.gpsimd.add_instruction(
                mybir.InstCollectiveCompute(
                    name=f"I-{nc.next_id()}",
                    kind="ReduceScatter",
                    op=mybir.AluOpType.add,
                    replica_groups=rgs,
                    ins=[nc.gpsimd.lower_ap(rsin[:], opt=False)],
                    outs=[nc.gpsimd.lower_ap(rsout[:], opt=False)],
                    unique_tensors="No",
                    cc_dim="Partition",
                )
            )
            nc.has_collectives = True

            # --- finalize: g scale + bias -----------------------------
            rs_sb = cpool.tile([128, NWJ, PC], F16)
            rso_perm = bass.AP(
                tensor=rsout.ap().tensor, offset=0,
                ap=[[PC, 128], [128 * PC, NWJ], [1, PC]],
            )
            nc.sync.dma_start(rs_sb[:], rso_perm)
            # tiny parallel load of just the col_deg column so the g chain
            # overlaps the big payload load
            cd_sb = cpool.tile([128, NWJ], F16)
            cd_perm = bass.AP(
                tensor=rsout.ap().tensor, offset=OC,
                ap=[[PC, 128], [128 * PC, NWJ], [1, 1]],
            )
            nc.scalar.dma_start(cd_sb[:], cd_perm)
            # empty dst columns have zero partials, so g needs no mask.
            g1 = cpool.tile([128, NWJ], F32)
            nc.vector.tensor_scalar(
                out=g1[:], in0=cd_sb[:], scalar1=1.0, scalar2=None,
                op0=mybir.AluOpType.max,
            )
            nc.scalar.sqrt(g1[:], g1[:])
            g_sb = cpool.tile([128, NWJ], F32)
            nc.vector.reciprocal(g_sb[:], g1[:])
            ogs = [wpool.tile([128, OC], F16, tag="og", bufs=NWJ, name=f"og{v}")
                   for v in range(NWJ)]
            for v in range(NWJ):
                nc.scalar.mul(ogs[v][:], rs_sb[:, v, 0:OC], g_sb[:, v:v + 1])
            for v in range(NWJ):
                nc.vector.tensor_tensor(
                    out=ogs[v][:], in0=ogs[v][:], in1=brep_sb[:],
                    op=mybir.AluOpType.add,
                )
            for v in range(NWJ):
                nc.sync.dma_start(out[v * 128:(v + 1) * 128, :], ogs[v][:])

    nc.finalize()
    return nc


def make_in_maps(x, weight, bias, edge_index):
    """Host-side sharding/layout only: grouping, dedup and padding of the
    edge list (index preprocessing); no arithmetic on float tensor data."""
    x = np.asarray(x, dtype=np.float32)
    weight = np.ascontiguousarray(np.asarray(weight, dtype=np.float32))
    bias = np.asarray(bias, dtype=np.float32)
    ei = np.asarray(edge_index)
    s_all = ei[0].astype(np.int64)
    d_all = ei[1].astype(np.int64)
    assert s_all.min() >= 0 and s_all.max() < ND, "src ids out of supported range"
    assert d_all.min() >= 0 and d_all.max() < ND, "dst ids out of supported range"

    brep = np.ascontiguousarray(np.tile(bias[None, :], (128, 1)).astype(np.float16))
    whr = np.ascontiguousarray(weight.reshape(IC // 128, 128, OC).transpose(1, 0, 2))

    core_of = s_all >> 9
    in_maps = []
    for c in range(CORES):
        m = core_of == c
        gl = s_all[m] - c * GSH           # local gene id [0, 512)
        d = d_all[m]                      # full dst id [0, 4000)
        key = gl * NDP + d
        uniq, cnt = np.unique(key, return_counts=True)
        gu = uniq // NDP
        du = uniq % NDP
        j = gu >> 7
        p = gu & 127
        q = du >> 10
        dloc = du & (QW - 1)
        # slot rank within each (j, q, p) group (uniq sorted => groups together)
        gk = (gu << 2) | q
        _, start_idx = np.unique(gk, return_index=True)
        starts = np.zeros(len(gk), dtype=np.int64)
        starts[start_idx] = start_idx
        starts = np.maximum.accumulate(starts)
        rank = np.arange(len(gk)) - starts
        assert rank.max() < KMAX, f"slot overflow: {rank.max() + 1} > {KMAX}"

        idx_t = np.full((128, NWJ * NQ * KMAX), -1, dtype=np.int16)
        dat_t = np.zeros((128, NWJ * NQ * KMAX), dtype=np.float16)
        col = j * (NQ * KMAX) + q * KMAX + rank
        idx_t[p, col] = dloc.astype(np.int16)
        dat_t[p, col] = cnt.astype(np.float16)

        xs = np.zeros((GSH, IC), dtype=np.float32)
        n = min(GSH, ND - c * GSH)
        xs[:n] = x[c * GSH:c * GSH + n]
        xhr = np.ascontiguousarray(xs.T.reshape(IC // 128, 128, GSH).transpose(1, 0, 2))

        in_maps.append(
            {
                "xh": xhr,
                "wh": whr,
                "brep": brep,
                "idxt": idx_t,
                "datt": dat_t.reshape(128, NWJ, NQ * KMAX),
            }
        )
    return in_maps


_NC = None


def _get_nc():
    global _NC
    if _NC is None:
        _NC = build_nc()
    return _NC


def kernel(x, weight, bias, edge_index, **run_kwargs):
    from concourse.bass_utils import run_bass_kernel_spmd

    nc = _get_nc()
    in_maps = make_in_maps(x, weight, bias, edge_index)
    res = run_bass_kernel_spmd(nc, in_maps, core_ids=list(range(CORES)), **run_kwargs)
    outs = res.results if hasattr(res, "results") else res
    full = np.empty((ND, OC), dtype=np.float32)
    for c in range(CORES):
        n = min(GSH, ND - c * GSH)
        full[c * GSH:c * GSH + n] = outs[c]["out"][:n].astype(np.float32)
    if run_kwargs:
        return full, res
    return full
